# revision 4
# baseline (speedup 1.0000x reference)
"""CARAFE-naive upsampling (N=4, C=256, H=W=64, k=5, g=4, s=2) on 8 TRN2
NeuronCores.

Sharding: core c <- (batch n = c//2, group-pair j = c%2): 128 feature
channels (2 of the 4 mask groups) of one batch image per core.

Banded-matmul formulation (see v1): per source row r and w-tile of Wt=32
columns, out[(g,c), (w,a,b)] += sum_{w'} statT[(g,w'), (g,c)] * B[(g,w'),
(w,a,b)], where B holds mask values on shifted diagonals and the 5 kernel
rows di accumulate into PSUM over r = h + di - 2.

v3, tuned from v2's trace (v2 = 101.7us, PE 70us busy at 78ns/MM from
un-hidden per-MM LDWEIGHTS; DMA engines 0-9 pinned at 100% for 70us
because 40-partition loads only reach 10 of 16 SDMA engines):
- Wt=32, K=72 exactly: loads span partitions 0-71 -> all 16 engines
  (partition p maps to engine (p//4) mod 16). No K padding: no FWL, but
  no memsets/pad shipping either; LDWEIGHTS (~107ns) hides under the
  larger matmul streams.
- h-QUAD psum packing: 4 adjacent output row-pairs share each (r, t)
  stationary with di differing by 1 -> one matmul with an affine hq-step
  of (hq_stride - di_stride) covers up to 4 rows (N up to 512 = one full
  PSUM bank). ~9 MMs per r instead of v1's 10 small ones, most N>=256.
- B ships as fp8 E3M4 (4 mantissa bits): masks are U[0,1); measured
  end-to-end rel err 1.27% vs the 2% gate (bf16 everything else).
  Mixed-dtype matmul (bf16 stationary x fp8 moving) is HW-supported.
- bytes/core: bmat 5.9 MB + stat 1.18 + out 4.19 = 11.3 MB (v1: 21).

v4 on v3's trace (v3 = 89.2us, PE 67.5us busy at 197ns/MM): K=72 lost
FWL, and each matmul's redundant LDWEIGHTS (~107ns, same row groups as
the in-flight MM) serializes with the stream. v4 pads K back to 128:
FWL + background-buffer overlap brings the per-MM weight-load cost to
~28ns (v1 measured 81ns/MM at N=128). Pad rows [72:128) of each B slot
are memset ONCE — the DMA never writes them, so they stay finite for
the whole kernel (they face exactly-zero stat rows, but fp8 garbage
could encode NaN and 0*NaN=NaN). stat ships compact per-group and is
scattered into the block-diagonal layout on device (full-tile memset +
2 rectangular DMAs per chunk).
"""

import sys

import numpy as np

for _p in ("/opt/trn_rl_repo", "/opt/pypackages"):
    if _p not in sys.path:
        sys.path.append(_p)

import ml_dtypes  # noqa: E402
from contextlib import ExitStack  # noqa: E402

import concourse.bass as bass  # noqa: E402
import concourse.tile as tile  # noqa: E402
from concourse import bacc, mybir  # noqa: E402
from concourse.bass_utils import run_bass_kernel_spmd  # noqa: E402

KS = 5            # kernel size
S = 2             # upscale
N, C, H, W = 4, 256, 64, 64
Wt = 32           # w-tile
NT = W // Wt      # 2 tiles
KB = Wt + 4       # band rows per group = 36
KK = 2 * KB       # contraction dim = 72
NQ = H // 4       # 16 h-quads
HFREE = Wt * KS * NT * S * S       # per-h free elems = 1280
BFREE = 4 * HFREE                  # quad tile free elems = 5120
BF16 = ml_dtypes.bfloat16
FP8 = ml_dtypes.float8_e3m4

_NC_CACHE = {}


def _build_bass():
    nc = bacc.Bacc()
    # banded masks, one tile per h-quad, fp8 E3M4, matmul-ready:
    #   bmat[Q, g*KB + w + dj, hq, w, di, t, a, b] = m[g,di,dj,4Q+hq,a,t,w,b]
    bmat_d = nc.declare_dram_parameter(
        "bmat", [NQ, KK, BFREE], mybir.dt.float8e3, isOutput=False)
    # compact per-group stationary:
    #   statc[g, w', r, t, cc] = fpad[g*64+cc, r, 32t+w']
    statc_d = nc.declare_dram_parameter(
        "statc", [2, KB, H, NT, 64], mybir.dt.bfloat16, isOutput=False)
    out_d = nc.declare_dram_parameter(
        "out", [128, S * H, S * W], mybir.dt.bfloat16, isOutput=True)

    NSLOT = 16   # one slot per quad (5 KB/partition each, fp8): no slot
                 # reuse, so every tile's pad rows get their own memset
    HB = 4       # output rows per batched store (= 1 quad)
    out_rows = out_d.rearrange("c (hb y) x -> c hb (y x)", hb=H // HB)

    # B tile free strides (elems): [hq, w, di, t, a, b]
    ST_HQ = HFREE                  # 1280
    ST_W = KS * NT * S * S         # 40
    ST_DI = NT * S * S             # 8
    ST_T = S * S                   # 4

    with tile.TileContext(nc) as tc, ExitStack() as ctx:
        statp = ctx.enter_context(tc.tile_pool(name="statp", bufs=1))
        bp = ctx.enter_context(tc.tile_pool(name="bp", bufs=NSLOT))
        pp = ctx.enter_context(tc.tile_pool(name="pp", bufs=4, space="PSUM"))
        op = ctx.enter_context(tc.tile_pool(name="op", bufs=3))

        btiles = {}
        psums = {}
        started = {}   # quad -> set of started t-banks (bank = t)

        def load_b(q):
            # K padded to 128 for FWL; rows [KK:128) face zero stat rows
            # but must be FINITE (fp8 garbage can encode NaN), so each
            # slot's pad rows are memset once — the DMA never writes
            # them, so the zeros persist across slot reuse.
            bt = bp.tile([128, 4, Wt, KS, NT, S, S], mybir.dt.float8e3,
                         name=f"bt{q}", tag="bt")
            if q < NSLOT:
                # memset duration scales with free elems, so split the
                # free dim (not partitions) across the two idle engines
                # engine ops need 32-aligned start partitions; rows
                # [64:KK) get overwritten by the DMA afterwards
                nc.vector.memset(bt[64:128, 0:2], 0.0)
                nc.gpsimd.memset(bt[64:128, 2:4], 0.0)
            eng = nc.sync if q % 2 == 0 else nc.scalar
            eng.dma_start(out=bt[0:KK], in_=bmat_d[q])
            btiles[q] = bt

        stats = [None] * (H // 4)

        def load_stat(rb, eng):
            st = statp.tile([128, 4, NT, 128], mybir.dt.bfloat16,
                            name=f"st{rb}", tag=f"st{rb}")
            # zeros: block off-diagonal + K-pad rows [KK:128)
            (nc.gpsimd if rb % 2 else nc.vector).memset(st, 0.0)
            for g in range(2):
                eng.dma_start(
                    out=st[g * KB:(g + 1) * KB, :, :, g * 64:(g + 1) * 64],
                    in_=statc_d[g, :, 4 * rb: 4 * rb + 4])
            stats[rb] = st

        load_stat(0, nc.scalar)  # gates MM #1 — first on its queue
        load_b(0)
        load_stat(1, nc.sync)
        for q in range(1, 6):  # warm a few B slots; r-loop prefetch does
                               # the rest in first-use order
            load_b(q)

        def last_r(q):
            return min(H - 1, 4 * q + 5)

        for r in range(H):
            # prefetch B quad-tiles ahead of the live window
            for q in range(max(0, (r - 2) // 4), min(NQ - 1, r // 4 + 5) + 1):
                if q not in btiles:
                    load_b(q)
            rb_need = min(H // 4 - 1, (r + 10) // 4)
            if stats[rb_need] is None:
                load_stat(rb_need, nc.sync if rb_need % 2 else nc.scalar)

            # live quads at this r: rows [r-2, r+2] clipped
            lo, hi = max(0, r - 2), min(H - 1, r + 2)
            for t in range(NT):
                for q in range(lo // 4, hi // 4 + 1):
                    # rows of quad q live at this r, as hq range
                    h0, h1 = max(lo, 4 * q), min(hi, 4 * q + 3)
                    if q not in psums:
                        psums[q] = pp.tile(
                            [128, NT, 4, Wt, S, S], mybir.dt.float32,
                            name=f"ps{q}", tag="ps")
                        started[q] = set()
                    ps = psums[q]
                    st = stats[r // 4][:, r % 4, t, :]
                    bt = btiles[q]
                    first = t not in started[q]
                    started[q].add(t)
                    stop = (r == last_r(q) and t == NT - 1)
                    # rows h0..h1 use di = r+2-h; fresh row (first touch at
                    # this r) is h = r+2 (di=0) — its PSUM bytes are still
                    # pending-zero, so it must be touched by its own MM
                    # (CoreSim requires uniform regions; HW would be fine).
                    nh = h1 - h0 + 1
                    fresh = (h1 == r + 2)
                    if fresh and nh > 1:
                        blocks = [(h0, nh - 1), (h1, 1)]
                    else:
                        blocks = [(h0, nh)]
                    for hb0, cnt in blocks:
                        di0 = r + 2 - hb0   # di of first row in block
                        if cnt == 1:
                            rhs = bt[:, hb0 % 4, :, di0, t, :, :]
                        else:
                            rhs = bass.AP(
                                tensor=bt.tensor,
                                offset=(bt.offset + (hb0 % 4) * ST_HQ
                                        + di0 * ST_DI + t * ST_T),
                                ap=[[BFREE, 128], [ST_HQ - ST_DI, cnt],
                                    [ST_W, Wt], [S, S], [1, S]],
                            )
                        nc.tensor.matmul(
                            out=ps[:, t, hb0 % 4: hb0 % 4 + cnt],
                            lhsT=st, rhs=rhs,
                            start=first, stop=stop and hb0 + cnt - 1 == h1,
                            skip_group_check=True)
                        first = False

            # drain quad q when its last contribution was at r-1
            done = [(r - 6) // 4] if (r >= 6 and (r - 6) % 4 == 0) else []
            if r == H - 1:
                done += [NQ - 1]
            for q in done:
                ot = op.tile([128, HB, S, NT, Wt, S], mybir.dt.bfloat16,
                             name=f"ot{q}", tag="ot")
                for hq in range(4):
                    # ot row layout (a, t, w, b); psum is (t, hq, w, a, b)
                    nc.vector.tensor_copy(
                        out=ot[:, hq],
                        in_=psums[q][:, :, hq].rearrange(
                            "c t w a b -> c a t w b"))
                del psums[q], btiles[q]
                del started[q]
                eng = nc.sync if q % 2 == 0 else nc.scalar
                eng.dma_start(out=out_rows[:, q], in_=ot)

    nc.finalize()
    return nc


def _host_shards(features, masks):
    """Build per-core stat (bf16) / bmat (fp8 e3m4) arrays."""
    in_maps = []
    iw = np.arange(Wt)
    for c in range(8):
        n, j = c // 2, c % 2
        f = features[n, 128 * j: 128 * (j + 1)]        # [128, 64, 64] f32
        m = masks[n, 50 * j: 50 * j + 50]              # [50, 128, 128] f32

        # statc[g, w', r, t, cc] = fpad[g*64+cc, r, 32t+w']
        statc = np.empty((2, KB, H, NT, 64), np.float32)
        fp = np.pad(f, ((0, 0), (0, 0), (2, 2)))
        for g in range(2):
            for t in range(NT):
                sl = fp[g * 64:(g + 1) * 64, :, Wt * t: Wt * t + KB]
                statc[g, :, :, t, :] = sl.transpose(2, 1, 0)

        # B[Q, g*KB + w + dj, hq, w, di, t, a, b] = m[g,di,dj,4Q+hq,a,t,w,b]
        M8 = m.reshape(2, KS, KS, H, S, NT, Wt, S)     # g,di,dj,h,a,t,w,b
        B = np.zeros((NQ, KK, 4, Wt, KS, NT, S, S), np.float32)
        for g in range(2):
            for dj in range(KS):
                src = M8[g, :, dj].reshape(KS, NQ, 4, S, NT, Wt, S)
                # dest adv-index dims: [w, Q, hq, di, t, a, b]
                B[:, g * KB + dj + iw, :, iw] = src.transpose(5, 1, 2, 0, 4, 3, 6)

        in_maps.append({
            "statc": np.ascontiguousarray(statc).astype(BF16),
            "bmat": np.ascontiguousarray(B).reshape(NQ, KK, BFREE).astype(FP8),
        })
    return in_maps


def kernel(features, masks, _trace=False):
    features = np.asarray(features, dtype=np.float32)
    masks = np.asarray(masks, dtype=np.float32)

    in_maps = _host_shards(features, masks)

    if "nc" not in _NC_CACHE:
        _NC_CACHE["nc"] = _build_bass()
    nc = _NC_CACHE["nc"]

    res = run_bass_kernel_spmd(nc, in_maps, list(range(8)), trace=_trace)
    kernel._last_result = res

    out = np.empty((N, C, S * H, S * W), np.float32)
    for c in range(8):
        n, j = c // 2, c % 2
        out[n, 128 * j: 128 * (j + 1)] = \
            res.results[c]["out"].astype(np.float32)
    return out


# revision 6
# speedup vs baseline: 1.3699x; 1.3699x over previous
"""CARAFE-naive upsampling (N=4, C=256, H=W=64, k=5, g=4, s=2) on 8 TRN2
NeuronCores.

Sharding: core c <- (batch n = c//2, group-pair j = c%2): 128 feature
channels (2 of the 4 mask groups) of one batch image per core.

Banded-matmul formulation (see v1): per source row r and w-tile of Wt=32
columns, out[(g,c), (w,a,b)] += sum_{w'} statT[(g,w'), (g,c)] * B[(g,w'),
(w,a,b)], where B holds mask values on shifted diagonals and the 5 kernel
rows di accumulate into PSUM over r = h + di - 2.

v3, tuned from v2's trace (v2 = 101.7us, PE 70us busy at 78ns/MM from
un-hidden per-MM LDWEIGHTS; DMA engines 0-9 pinned at 100% for 70us
because 40-partition loads only reach 10 of 16 SDMA engines):
- Wt=32, K=72 exactly: loads span partitions 0-71 -> all 16 engines
  (partition p maps to engine (p//4) mod 16). No K padding: no FWL, but
  no memsets/pad shipping either; LDWEIGHTS (~107ns) hides under the
  larger matmul streams.
- h-QUAD psum packing: 4 adjacent output row-pairs share each (r, t)
  stationary with di differing by 1 -> one matmul with an affine hq-step
  of (hq_stride - di_stride) covers up to 4 rows (N up to 512 = one full
  PSUM bank). ~9 MMs per r instead of v1's 10 small ones, most N>=256.
- B ships as fp8 E3M4 (4 mantissa bits): masks are U[0,1); measured
  end-to-end rel err 1.27% vs the 2% gate (bf16 everything else).
  Mixed-dtype matmul (bf16 stationary x fp8 moving) is HW-supported.
- bytes/core: bmat 5.9 MB + stat 1.18 + out 4.19 = 11.3 MB (v1: 21).

v4 on v3's trace (v3 = 89.2us, PE 67.5us busy at 197ns/MM): K=72 lost
FWL, and each matmul's redundant LDWEIGHTS (~107ns, same row groups as
the in-flight MM) serializes with the stream. v4 pads K back to 128:
FWL + background-buffer overlap brings the per-MM weight-load cost to
~28ns (v1 measured 81ns/MM at N=128). Pad rows [72:128) of each B slot
are memset ONCE — the DMA never writes them, so they stay finite for
the whole kernel (they face exactly-zero stat rows, but fp8 garbage
could encode NaN and 0*NaN=NaN). stat ships compact per-group and is
scattered into the block-diagonal layout on device (full-tile memset +
2 rectangular DMAs per chunk).
"""

import sys

import numpy as np

for _p in ("/opt/trn_rl_repo", "/opt/pypackages"):
    if _p not in sys.path:
        sys.path.append(_p)

import ml_dtypes  # noqa: E402
from contextlib import ExitStack  # noqa: E402

import concourse.bass as bass  # noqa: E402
import concourse.tile as tile  # noqa: E402
from concourse import bacc, mybir  # noqa: E402
from concourse.bass_utils import run_bass_kernel_spmd  # noqa: E402

KS = 5            # kernel size
S = 2             # upscale
N, C, H, W = 4, 256, 64, 64
Wt = 32           # w-tile
NT = W // Wt      # 2 tiles
KB = Wt + 4       # band rows per group = 36
KK = 2 * KB       # contraction dim = 72
NQ = H // 4       # 16 h-quads
HFREE = Wt * KS * NT * S * S       # per-h free elems = 1280
BFREE = 4 * HFREE                  # quad tile free elems = 5120
BF16 = ml_dtypes.bfloat16
FP8 = ml_dtypes.float8_e3m4

_NC_CACHE = {}


def _build_bass():
    nc = bacc.Bacc()
    # banded masks, one tile per h-quad, fp8 E3M4, matmul-ready:
    #   bmat[Q, g*KB + w + dj, hq, w, di, t, a, b] = m[g,di,dj,4Q+hq,a,t,w,b]
    bmat_d = nc.declare_dram_parameter(
        "bmat", [NQ, KK, BFREE], mybir.dt.float8e3, isOutput=False)
    # stationary (dense 72 rows, block-diag zeros included — compact
    # per-group shipping fragments the DMA into 128-B descriptors):
    #   stat[g*KB + w', r, t, g*64 + cc] = fpad[g*64+cc, r, 32t+w']
    stat_d = nc.declare_dram_parameter(
        "stat", [KK, H, NT, 128], mybir.dt.bfloat16, isOutput=False)
    out_d = nc.declare_dram_parameter(
        "out", [128, S * H, S * W], mybir.dt.bfloat16, isOutput=True)

    NSLOT = 16   # one slot per quad (5 KB/partition each, fp8): no slot
                 # reuse, so every tile's pad rows get their own memset
    HB = 4       # output rows per batched store (= 1 quad)
    out_rows = out_d.rearrange("c (hb y) x -> c hb (y x)", hb=H // HB)

    # B tile free strides (elems): [hq, w, di, t, a, b]
    ST_HQ = HFREE                  # 1280
    ST_W = KS * NT * S * S         # 40
    ST_DI = NT * S * S             # 8
    ST_T = S * S                   # 4

    with tile.TileContext(nc) as tc, ExitStack() as ctx:
        statp = ctx.enter_context(tc.tile_pool(name="statp", bufs=1))
        bp = ctx.enter_context(tc.tile_pool(name="bp", bufs=NSLOT))
        pp = ctx.enter_context(tc.tile_pool(name="pp", bufs=4, space="PSUM"))
        op = ctx.enter_context(tc.tile_pool(name="op", bufs=3))

        btiles = {}
        psums = {}
        started = {}   # quad -> set of started t-banks (bank = t)

        def load_b(q):
            # K padded to 128 for FWL; rows [KK:128) face zero stat rows
            # but must be FINITE (fp8 garbage can encode NaN), so each
            # slot's pad rows are memset once — the DMA never writes
            # them, so the zeros persist across slot reuse.
            # (a,b) merged into one dim of 4 so views bitcast to f32
            bt = bp.tile([128, 4, Wt, KS, NT, S * S], mybir.dt.float8e3,
                         name=f"bt{q}", tag="bt")
            if q < NSLOT:
                # memset duration scales with free elems, so split the
                # free dim (not partitions) across the two idle engines
                # engine ops need 32-aligned start partitions; rows
                # [64:KK) get overwritten by the DMA afterwards. f32
                # bitcast: memset runs at 1 elem/cycle regardless of
                # dtype width, so wider elems = 4x fewer cycles.
                nc.vector.memset(
                    bt[64:128, 0:2].bitcast(mybir.dt.float32), 0.0)
                nc.gpsimd.memset(
                    bt[64:128, 2:4].bitcast(mybir.dt.float32), 0.0)
            eng = nc.sync if q % 2 == 0 else nc.scalar
            eng.dma_start(out=bt[0:KK], in_=bmat_d[q])
            btiles[q] = bt

        stats = [None] * (H // 4)

        def load_stat(rb, eng):
            st = statp.tile([128, 4, NT, 128], mybir.dt.bfloat16,
                            name=f"st{rb}", tag=f"st{rb}")
            # K-pad rows [KK:128) must be exactly 0 (they multiply the
            # B tiles' garbage pad rows); [64:KK) is overwritten by the
            # DMA. f32 bitcast halves the memset cycles.
            nc.gpsimd.memset(st[64:128].bitcast(mybir.dt.float32), 0.0)
            eng.dma_start(out=st[0:KK], in_=stat_d[:, 4 * rb: 4 * rb + 4])
            stats[rb] = st

        load_stat(0, nc.scalar)  # gates MM #1 — first on its queue
        load_b(0)
        load_stat(1, nc.sync)
        for q in range(1, 6):  # warm a few B slots; r-loop prefetch does
                               # the rest in first-use order
            load_b(q)

        def last_r(q):
            return min(H - 1, 4 * q + 5)

        for r in range(H):
            # prefetch B quad-tiles ahead of the live window
            for q in range(max(0, (r - 2) // 4), min(NQ - 1, r // 4 + 5) + 1):
                if q not in btiles:
                    load_b(q)
            rb_need = min(H // 4 - 1, (r + 10) // 4)
            if stats[rb_need] is None:
                load_stat(rb_need, nc.sync if rb_need % 2 else nc.scalar)

            # live quads at this r: rows [r-2, r+2] clipped
            lo, hi = max(0, r - 2), min(H - 1, r + 2)
            for t in range(NT):
                for q in range(lo // 4, hi // 4 + 1):
                    # rows of quad q live at this r, as hq range
                    h0, h1 = max(lo, 4 * q), min(hi, 4 * q + 3)
                    if q not in psums:
                        psums[q] = pp.tile(
                            [128, NT, 4, Wt, S, S], mybir.dt.float32,
                            name=f"ps{q}", tag="ps")
                        started[q] = set()
                    ps = psums[q]
                    st = stats[r // 4][:, r % 4, t, :]
                    bt = btiles[q]
                    first = t not in started[q]
                    started[q].add(t)
                    stop = (r == last_r(q) and t == NT - 1)
                    # rows h0..h1 use di = r+2-h; fresh row (first touch at
                    # this r) is h = r+2 (di=0) — its PSUM bytes are still
                    # pending-zero, so it must be touched by its own MM
                    # (CoreSim requires uniform regions; HW would be fine).
                    nh = h1 - h0 + 1
                    fresh = (h1 == r + 2)
                    if fresh and nh > 1:
                        blocks = [(h0, nh - 1), (h1, 1)]
                    else:
                        blocks = [(h0, nh)]
                    for hb0, cnt in blocks:
                        di0 = r + 2 - hb0   # di of first row in block
                        if cnt == 1:
                            rhs = bt[:, hb0 % 4, :, di0, t, :]
                        else:
                            rhs = bass.AP(
                                tensor=bt.tensor,
                                offset=(bt.offset + (hb0 % 4) * ST_HQ
                                        + di0 * ST_DI + t * ST_T),
                                ap=[[BFREE, 128], [ST_HQ - ST_DI, cnt],
                                    [ST_W, Wt], [1, S * S]],
                            )
                        nc.tensor.matmul(
                            out=ps[:, t, hb0 % 4: hb0 % 4 + cnt],
                            lhsT=st, rhs=rhs,
                            start=first, stop=stop and hb0 + cnt - 1 == h1,
                            skip_group_check=True)
                        first = False

            # drain quad q when its last contribution was at r-1
            done = [(r - 6) // 4] if (r >= 6 and (r - 6) % 4 == 0) else []
            if r == H - 1:
                done += [NQ - 1]
            for q in done:
                ot = op.tile([128, HB, S, NT, Wt, S], mybir.dt.bfloat16,
                             name=f"ot{q}", tag="ot")
                for hq in range(4):
                    # ot row layout (a, t, w, b); psum is (t, hq, w, a, b)
                    # (ACT only takes 3 free AP dims, so DVE does all of
                    # these strided 4-dim copies)
                    nc.vector.tensor_copy(
                        out=ot[:, hq],
                        in_=psums[q][:, :, hq].rearrange(
                            "c t w a b -> c a t w b"))
                del psums[q], btiles[q]
                del started[q]
                eng = nc.sync if q % 2 == 0 else nc.scalar
                eng.dma_start(out=out_rows[:, q], in_=ot)

    nc.finalize()
    return nc


def _host_shards(features, masks):
    """Build per-core stat (bf16) / bmat (fp8 e3m4) arrays."""
    in_maps = []
    iw = np.arange(Wt)
    for c in range(8):
        n, j = c // 2, c % 2
        f = features[n, 128 * j: 128 * (j + 1)]        # [128, 64, 64] f32
        m = masks[n, 50 * j: 50 * j + 50]              # [50, 128, 128] f32

        # stat[g*KB + w', r, t, g*64+cc] = fpad[g*64+cc, r, 32t+w']
        stat = np.zeros((KK, H, NT, 128), np.float32)
        fp = np.pad(f, ((0, 0), (0, 0), (2, 2)))
        for g in range(2):
            for t in range(NT):
                sl = fp[g * 64:(g + 1) * 64, :, Wt * t: Wt * t + KB]
                stat[g * KB:(g + 1) * KB, :, t, g * 64:(g + 1) * 64] = \
                    sl.transpose(2, 1, 0)

        # B[Q, g*KB + w + dj, hq, w, di, t, a, b] = m[g,di,dj,4Q+hq,a,t,w,b]
        M8 = m.reshape(2, KS, KS, H, S, NT, Wt, S)     # g,di,dj,h,a,t,w,b
        B = np.zeros((NQ, KK, 4, Wt, KS, NT, S, S), np.float32)
        for g in range(2):
            for dj in range(KS):
                src = M8[g, :, dj].reshape(KS, NQ, 4, S, NT, Wt, S)
                # dest adv-index dims: [w, Q, hq, di, t, a, b]
                B[:, g * KB + dj + iw, :, iw] = src.transpose(5, 1, 2, 0, 4, 3, 6)

        in_maps.append({
            "stat": np.ascontiguousarray(stat).astype(BF16),
            "bmat": np.ascontiguousarray(B).reshape(NQ, KK, BFREE).astype(FP8),
        })
    return in_maps


def kernel(features, masks, _trace=False):
    features = np.asarray(features, dtype=np.float32)
    masks = np.asarray(masks, dtype=np.float32)

    in_maps = _host_shards(features, masks)

    if "nc" not in _NC_CACHE:
        _NC_CACHE["nc"] = _build_bass()
    nc = _NC_CACHE["nc"]

    res = run_bass_kernel_spmd(nc, in_maps, list(range(8)), trace=_trace)
    kernel._last_result = res

    out = np.empty((N, C, S * H, S * W), np.float32)
    for c in range(8):
        n, j = c // 2, c % 2
        out[n, 128 * j: 128 * (j + 1)] = \
            res.results[c]["out"].astype(np.float32)
    return out


# revision 7
# speedup vs baseline: 1.5572x; 1.1367x over previous
"""CARAFE-naive upsampling (N=4, C=256, H=W=64, k=5, g=4, s=2) on 8 TRN2
NeuronCores.

Sharding: core c <- (batch n = c//2, group-pair j = c%2): 128 feature
channels (2 of the 4 mask groups) of one batch image per core.

Banded-matmul formulation (see v1): per source row r and w-tile of Wt=32
columns, out[(g,c), (w,a,b)] += sum_{w'} statT[(g,w'), (g,c)] * B[(g,w'),
(w,a,b)], where B holds mask values on shifted diagonals and the 5 kernel
rows di accumulate into PSUM over r = h + di - 2.

v3, tuned from v2's trace (v2 = 101.7us, PE 70us busy at 78ns/MM from
un-hidden per-MM LDWEIGHTS; DMA engines 0-9 pinned at 100% for 70us
because 40-partition loads only reach 10 of 16 SDMA engines):
- Wt=32, K=72 exactly: loads span partitions 0-71 -> all 16 engines
  (partition p maps to engine (p//4) mod 16). No K padding: no FWL, but
  no memsets/pad shipping either; LDWEIGHTS (~107ns) hides under the
  larger matmul streams.
- h-QUAD psum packing: 4 adjacent output row-pairs share each (r, t)
  stationary with di differing by 1 -> one matmul with an affine hq-step
  of (hq_stride - di_stride) covers up to 4 rows (N up to 512 = one full
  PSUM bank). ~9 MMs per r instead of v1's 10 small ones, most N>=256.
- B ships as fp8 E3M4 (4 mantissa bits): masks are U[0,1); measured
  end-to-end rel err 1.27% vs the 2% gate (bf16 everything else).
  Mixed-dtype matmul (bf16 stationary x fp8 moving) is HW-supported.
- bytes/core: bmat 5.9 MB + stat 1.18 + out 4.19 = 11.3 MB (v1: 21).

v4 on v3's trace (v3 = 89.2us, PE 67.5us busy at 197ns/MM): K=72 lost
FWL, and each matmul's redundant LDWEIGHTS (~107ns, same row groups as
the in-flight MM) serializes with the stream. v4 pads K back to 128:
FWL + background-buffer overlap brings the per-MM weight-load cost to
~28ns (v1 measured 81ns/MM at N=128). Pad rows [72:128) of each B slot
are memset ONCE — the DMA never writes them, so they stay finite for
the whole kernel (they face exactly-zero stat rows, but fp8 garbage
could encode NaN and 0*NaN=NaN). stat ships compact per-group and is
scattered into the block-diagonal layout on device (full-tile memset +
2 rectangular DMAs per chunk).
"""

import sys

import numpy as np

for _p in ("/opt/trn_rl_repo", "/opt/pypackages"):
    if _p not in sys.path:
        sys.path.append(_p)

import ml_dtypes  # noqa: E402
from contextlib import ExitStack  # noqa: E402

import concourse.bass as bass  # noqa: E402
import concourse.tile as tile  # noqa: E402
from concourse import bacc, mybir  # noqa: E402
from concourse.bass_utils import run_bass_kernel_spmd  # noqa: E402

KS = 5            # kernel size
S = 2             # upscale
N, C, H, W = 4, 256, 64, 64
Wt = 32           # w-tile
NT = W // Wt      # 2 tiles
KB = Wt + 4       # band rows per group = 36
KK = 2 * KB       # contraction dim = 72
NQ = H // 4       # 16 h-quads
HFREE = Wt * KS * NT * S * S       # per-h free elems = 1280
BFREE = 4 * HFREE                  # quad tile free elems = 5120
BF16 = ml_dtypes.bfloat16
FP8 = ml_dtypes.float8_e3m4

_NC_CACHE = {}
_SIM_SPLIT = False   # set True for CoreSim runs (uniform-PSUM assert)


def _build_bass():
    nc = bacc.Bacc()
    # banded masks, one tile per h-quad, fp8 E3M4, matmul-ready:
    #   bmat[Q, g*KB + w + dj, hq, w, di, t, a, b] = m[g,di,dj,4Q+hq,a,t,w,b]
    bmat_d = nc.declare_dram_parameter(
        "bmat", [NQ, KK, BFREE], mybir.dt.float8e3, isOutput=False)
    # stationary (dense 72 rows, block-diag zeros included — compact
    # per-group shipping fragments the DMA into 128-B descriptors):
    #   stat[g*KB + w', r, t, g*64 + cc] = fpad[g*64+cc, r, 32t+w']
    stat_d = nc.declare_dram_parameter(
        "stat", [KK, H, NT, 128], mybir.dt.bfloat16, isOutput=False)
    out_d = nc.declare_dram_parameter(
        "out", [128, S * H, S * W], mybir.dt.bfloat16, isOutput=True)

    NSLOT = 16   # one slot per quad (5 KB/partition each, fp8): no slot
                 # reuse, so every tile's pad rows get their own memset
    HB = 4       # output rows per batched store (= 1 quad)
    out_rows = out_d.rearrange("c (hb y) x -> c hb (y x)", hb=H // HB)

    # B tile free strides (elems): [hq, w, di, t, a, b]
    ST_HQ = HFREE                  # 1280
    ST_W = KS * NT * S * S         # 40
    ST_DI = NT * S * S             # 8
    ST_T = S * S                   # 4

    with tile.TileContext(nc) as tc, ExitStack() as ctx:
        statp = ctx.enter_context(tc.tile_pool(name="statp", bufs=1))
        bp = ctx.enter_context(tc.tile_pool(name="bp", bufs=NSLOT))
        pp = ctx.enter_context(tc.tile_pool(name="pp", bufs=4, space="PSUM"))
        op = ctx.enter_context(tc.tile_pool(name="op", bufs=3))

        btiles = {}
        psums = {}
        started = {}   # quad -> set of started t-banks (bank = t)

        def load_b(q):
            # K padded to 128 for FWL; rows [KK:128) face zero stat rows
            # but must be FINITE (fp8 garbage can encode NaN), so each
            # slot's pad rows are memset once — the DMA never writes
            # them, so the zeros persist across slot reuse.
            # (a,b) merged into one dim of 4 so views bitcast to f32
            bt = bp.tile([128, 4, Wt, KS, NT, S * S], mybir.dt.float8e3,
                         name=f"bt{q}", tag="bt")
            if q < NSLOT:
                # memset duration scales with free elems, so split the
                # free dim (not partitions) across the two idle engines
                # engine ops need 32-aligned start partitions; rows
                # [64:KK) get overwritten by the DMA afterwards. f32
                # bitcast: memset runs at 1 elem/cycle regardless of
                # dtype width, so wider elems = 4x fewer cycles.
                # all memsets live on the otherwise-idle gpsimd: DVE
                # must never delay a PSUM drain (a stalled drain blocks
                # the TensorEngine on the psum pool)
                nc.gpsimd.memset(
                    bt[64:128].bitcast(mybir.dt.float32), 0.0)
            eng = nc.sync if q % 2 == 0 else nc.scalar
            eng.dma_start(out=bt[0:KK], in_=bmat_d[q])
            btiles[q] = bt

        stats = [None] * (H // 4)

        def load_stat(rb, eng):
            st = statp.tile([128, 4, NT, 128], mybir.dt.bfloat16,
                            name=f"st{rb}", tag=f"st{rb}")
            # K-pad rows [KK:128) must be exactly 0 (they multiply the
            # B tiles' garbage pad rows); [64:KK) is overwritten by the
            # DMA. f32 bitcast halves the memset cycles.
            nc.gpsimd.memset(st[64:128].bitcast(mybir.dt.float32), 0.0)
            eng.dma_start(out=st[0:KK], in_=stat_d[:, 4 * rb: 4 * rb + 4])
            stats[rb] = st

        load_stat(0, nc.scalar)  # gates MM #1 — first on its queue
        load_b(0)
        load_stat(1, nc.sync)
        for q in range(1, 6):  # warm a few B slots; r-loop prefetch does
                               # the rest in first-use order
            load_b(q)

        def last_r(q):
            return min(H - 1, 4 * q + 5)

        for r in range(H):
            # prefetch B quad-tiles ahead of the live window
            for q in range(max(0, (r - 2) // 4), min(NQ - 1, r // 4 + 5) + 1):
                if q not in btiles:
                    load_b(q)
            rb_need = min(H // 4 - 1, (r + 10) // 4)
            if stats[rb_need] is None:
                load_stat(rb_need, nc.sync if rb_need % 2 else nc.scalar)

            # live quads at this r: rows [r-2, r+2] clipped
            lo, hi = max(0, r - 2), min(H - 1, r + 2)
            for t in range(NT):
                for q in range(lo // 4, hi // 4 + 1):
                    # rows of quad q live at this r, as hq range
                    h0, h1 = max(lo, 4 * q), min(hi, 4 * q + 3)
                    if q not in psums:
                        psums[q] = pp.tile(
                            [128, NT, 4, Wt, S, S], mybir.dt.float32,
                            name=f"ps{q}", tag="ps")
                        started[q] = set()
                    ps = psums[q]
                    st = stats[r // 4][:, r % 4, t, :]
                    bt = btiles[q]
                    first = t not in started[q]
                    started[q].add(t)
                    stop = (r == last_r(q) and t == NT - 1)
                    # rows h0..h1 use di = r+2-h. A fresh row's PSUM
                    # bytes are pending-zero while older rows' are
                    # written; hardware start=False semantics are
                    # per-element (overwrite where pending, accumulate
                    # where written), so one MM may span both. CoreSim
                    # asserts uniform regions — _SIM_SPLIT re-enables the
                    # split for simulator runs (same arithmetic).
                    nh = h1 - h0 + 1
                    if _SIM_SPLIT and h1 == r + 2 and nh > 1:
                        blocks = [(h0, nh - 1), (h1, 1)]
                    else:
                        blocks = [(h0, nh)]
                    for hb0, cnt in blocks:
                        di0 = r + 2 - hb0   # di of first row in block
                        if cnt == 1:
                            rhs = bt[:, hb0 % 4, :, di0, t, :]
                        else:
                            rhs = bass.AP(
                                tensor=bt.tensor,
                                offset=(bt.offset + (hb0 % 4) * ST_HQ
                                        + di0 * ST_DI + t * ST_T),
                                ap=[[BFREE, 128], [ST_HQ - ST_DI, cnt],
                                    [ST_W, Wt], [1, S * S]],
                            )
                        nc.tensor.matmul(
                            out=ps[:, t, hb0 % 4: hb0 % 4 + cnt],
                            lhsT=st, rhs=rhs,
                            start=first, stop=stop and hb0 + cnt - 1 == h1,
                            skip_group_check=True)
                        first = False

            # drain quad q when its last contribution was at r-1
            done = [(r - 6) // 4] if (r >= 6 and (r - 6) % 4 == 0) else []
            if r == H - 1:
                done += [NQ - 1]
            for q in done:
                ot = op.tile([128, HB, S, NT, Wt, S], mybir.dt.bfloat16,
                             name=f"ot{q}", tag="ot")
                for hq in range(4):
                    # ot row layout (a, t, w, b); psum is (t, hq, w, a, b)
                    # (ACT only takes 3 free AP dims, so DVE does all of
                    # these strided 4-dim copies)
                    nc.vector.tensor_copy(
                        out=ot[:, hq],
                        in_=psums[q][:, :, hq].rearrange(
                            "c t w a b -> c a t w b"))
                del psums[q], btiles[q]
                del started[q]
                eng = nc.sync if q % 2 == 0 else nc.scalar
                eng.dma_start(out=out_rows[:, q], in_=ot)

    nc.finalize()
    return nc


def _host_shards(features, masks):
    """Build per-core stat (bf16) / bmat (fp8 e3m4) arrays."""
    in_maps = []
    iw = np.arange(Wt)
    for c in range(8):
        n, j = c // 2, c % 2
        f = features[n, 128 * j: 128 * (j + 1)]        # [128, 64, 64] f32
        m = masks[n, 50 * j: 50 * j + 50]              # [50, 128, 128] f32

        # stat[g*KB + w', r, t, g*64+cc] = fpad[g*64+cc, r, 32t+w']
        stat = np.zeros((KK, H, NT, 128), np.float32)
        fp = np.pad(f, ((0, 0), (0, 0), (2, 2)))
        for g in range(2):
            for t in range(NT):
                sl = fp[g * 64:(g + 1) * 64, :, Wt * t: Wt * t + KB]
                stat[g * KB:(g + 1) * KB, :, t, g * 64:(g + 1) * 64] = \
                    sl.transpose(2, 1, 0)

        # B[Q, g*KB + w + dj, hq, w, di, t, a, b] = m[g,di,dj,4Q+hq,a,t,w,b]
        M8 = m.reshape(2, KS, KS, H, S, NT, Wt, S)     # g,di,dj,h,a,t,w,b
        B = np.zeros((NQ, KK, 4, Wt, KS, NT, S, S), np.float32)
        for g in range(2):
            for dj in range(KS):
                src = M8[g, :, dj].reshape(KS, NQ, 4, S, NT, Wt, S)
                # dest adv-index dims: [w, Q, hq, di, t, a, b]
                B[:, g * KB + dj + iw, :, iw] = src.transpose(5, 1, 2, 0, 4, 3, 6)

        in_maps.append({
            "stat": np.ascontiguousarray(stat).astype(BF16),
            "bmat": np.ascontiguousarray(B).reshape(NQ, KK, BFREE).astype(FP8),
        })
    return in_maps


def kernel(features, masks, _trace=False):
    features = np.asarray(features, dtype=np.float32)
    masks = np.asarray(masks, dtype=np.float32)

    in_maps = _host_shards(features, masks)

    if "nc" not in _NC_CACHE:
        _NC_CACHE["nc"] = _build_bass()
    nc = _NC_CACHE["nc"]

    res = run_bass_kernel_spmd(nc, in_maps, list(range(8)), trace=_trace)
    kernel._last_result = res

    out = np.empty((N, C, S * H, S * W), np.float32)
    for c in range(8):
        n, j = c // 2, c % 2
        out[n, 128 * j: 128 * (j + 1)] = \
            res.results[c]["out"].astype(np.float32)
    return out
